# revision 3
# baseline (speedup 1.0000x reference)
"""Trainium2 Bass kernel for nn_KVCacheHybrid (quantized KV-cache scatter-update).

Reference semantics (per cache, k and v independently):
  1. 4-bit affine quantize along L (scales/zeros reduce over B,H,D per l)
  2. dequantize, scatter new rows at input_pos, re-quantize, dequantize.

Math shortcuts (proven by the earlier 243us baseline, rel err ~3e-4):
  * Second-pass min/max for non-updated l are the dequant grid endpoints:
    mn2 = z1 - 8*s1, mx2 = z1 + 7*s1 -> no second data reduction.
  * For non-updated l: out = q1 * s2 + mn2 with q1 = round((x - mn1)/s1).
  * Rows at input_pos depend only on k_val/v_val -> computed on host, spliced.

This version (vs that baseline):
  * l-major DRAM layout: host pre-transposes caches to [L, B*H*D] so every
    DMA line is 16KB contiguous (was 512B) -> full HBM rate, ~8x fewer
    descriptors, cheap triggers.
  * round folded into the ACT affine via the fp32->int8 output convert,
    which (measured on HW) is round-to-nearest-even with saturation --
    exactly jnp.round + clip.  Both elementwise passes run on ACT
    (fp32 -> int8 codes -> fp32), so DVE runs ONLY the min/max reductions
    and the per-l constant chain.
  * [128, 4096] half-row tiles, 6-deep input pool: the load->reduce->
    consts->act1 latency chain is ~3 tiles long, so 6 buffers keep the
    input DMA queue saturated (3 buffers measurably starved it).
  * act1 is gated on only inv1/nb1 (4 small ops after the reduces); the
    rest of the const chain is emitted later, feeding act3.
  * the 4 reduce-partial tiles live in a bufs=1 pool: the WAR hazard
    forces each chunk's min/max combines to run before the NEXT chunk's
    reductions (the FIFO-by-readiness scheduler otherwise defers them 2
    chunks, delaying act1 and starving the input DMA queue at startup).
    Measured 191781 ns vs 195877 ns without it.

Sharding: L axis across 8 cores (512 l's each, per-l reduction core-local,
no collectives).
"""

import numpy as np
from contextlib import ExitStack

import concourse.bass as bass
import concourse.bacc as bacc
import concourse.tile as tile
from concourse import mybir
from concourse.bass_utils import run_bass_kernel_spmd

F32 = mybir.dt.float32
I8 = mybir.dt.int8
ALU = mybir.AluOpType
AXIS = mybir.AxisListType
ACTF = mybir.ActivationFunctionType

B, H, L, D = 2, 32, 4096, 128
FD = B * H * D             # 8192 elements per l-row
N_CORES = 8
LC = L // N_CORES          # 512 l-rows per core
PCHUNK = 128               # l-rows per tile (partition dim)
TCOLS = 4096               # columns per tile; 2 col-tiles per l-row
C15 = float(np.float32(1.0 / 15.0))

_BUILD_CACHE = {}


def _build(lc=LC):
    nc = bacc.Bacc("TRN2", target_bir_lowering=False, debug=False,
                   num_devices=N_CORES)
    k = nc.dram_tensor("k", [lc, FD], F32, kind="ExternalInput").ap()
    v = nc.dram_tensor("v", [lc, FD], F32, kind="ExternalInput").ap()
    ok = nc.dram_tensor("ok", [lc, FD], F32, kind="ExternalOutput").ap()
    ov = nc.dram_tensor("ov", [lc, FD], F32, kind="ExternalOutput").ap()
    n_chunks = lc // PCHUNK

    with tile.TileContext(nc) as tc, ExitStack() as ctx:
        xpool = ctx.enter_context(tc.tile_pool(name="x", bufs=6))
        qpool = ctx.enter_context(tc.tile_pool(name="q", bufs=4))
        opool = ctx.enter_context(tc.tile_pool(name="o", bufs=4))
        cpool = ctx.enter_context(tc.tile_pool(name="c", bufs=2))
        ppool = ctx.enter_context(tc.tile_pool(name="p", bufs=1))

        for src, dst in ((k, ok), (v, ov)):
            for ch in range(n_chunks):
                l0 = ch * PCHUNK
                rows = src[l0:l0 + PCHUNK, :]
                XA = xpool.tile([PCHUNK, TCOLS], F32, tag="x")
                XB = xpool.tile([PCHUNK, TCOLS], F32, tag="x")
                nc.sync.dma_start(out=XA[:], in_=rows[:, 0:TCOLS])
                nc.sync.dma_start(out=XB[:], in_=rows[:, TCOLS:FD])

                mna = ppool.tile([PCHUNK, 1], F32, tag="mna")
                mnb = ppool.tile([PCHUNK, 1], F32, tag="mnb")
                mxa = ppool.tile([PCHUNK, 1], F32, tag="mxa")
                mxb = ppool.tile([PCHUNK, 1], F32, tag="mxb")
                nc.vector.tensor_reduce(mna[:], XA[:], axis=AXIS.X, op=ALU.min)
                nc.vector.tensor_reduce(mxa[:], XA[:], axis=AXIS.X, op=ALU.max)
                nc.vector.tensor_reduce(mnb[:], XB[:], axis=AXIS.X, op=ALU.min)
                nc.vector.tensor_reduce(mxb[:], XB[:], axis=AXIS.X, op=ALU.max)
                mn1 = cpool.tile([PCHUNK, 1], F32, tag="mn1")
                mx1 = cpool.tile([PCHUNK, 1], F32, tag="mx1")
                nc.vector.tensor_tensor(mn1[:], mna[:], mnb[:], op=ALU.min)
                nc.vector.tensor_tensor(mx1[:], mxa[:], mxb[:], op=ALU.max)

                # minimal chain for act1: dd -> s1 -> inv1 -> nb1
                dd = cpool.tile([PCHUNK, 1], F32, tag="dd")
                nc.vector.tensor_tensor(dd[:], mx1[:], mn1[:], op=ALU.subtract)
                s1 = cpool.tile([PCHUNK, 1], F32, tag="s1")
                nc.vector.tensor_scalar(s1[:], dd[:], 1e-6, C15,
                                        op0=ALU.max, op1=ALU.mult)
                inv1 = cpool.tile([PCHUNK, 1], F32, tag="inv1")
                nc.vector.reciprocal(inv1[:], s1[:])
                nb1 = cpool.tile([PCHUNK, 1], F32, tag="nb1")
                # nb1 = -(mn1 * inv1): bias of the quantize affine
                nc.vector.tensor_scalar(nb1[:], mn1[:], inv1[:, 0:1], -1.0,
                                        op0=ALU.mult, op1=ALU.mult)

                # q = RNE(x*inv1 + nb1) via the int8 output convert
                QA = qpool.tile([PCHUNK, TCOLS], I8, tag="q")
                QB = qpool.tile([PCHUNK, TCOLS], I8, tag="q")
                nc.scalar.activation(QA[:], XA[:], ACTF.Identity,
                                     bias=nb1[:, 0:1], scale=inv1[:, 0:1])
                nc.scalar.activation(QB[:], XB[:], ACTF.Identity,
                                     bias=nb1[:, 0:1], scale=inv1[:, 0:1])

                # rest of the const chain, feeding act3 only
                a8 = cpool.tile([PCHUNK, 1], F32, tag="a8")
                nc.vector.tensor_scalar(a8[:], s1[:], 8.0, None, op0=ALU.mult)
                z1 = cpool.tile([PCHUNK, 1], F32, tag="z1")
                nc.vector.tensor_tensor(z1[:], mn1[:], a8[:], op=ALU.add)
                mn2 = cpool.tile([PCHUNK, 1], F32, tag="mn2")
                nc.vector.tensor_tensor(mn2[:], z1[:], a8[:], op=ALU.subtract)
                b7 = cpool.tile([PCHUNK, 1], F32, tag="b7")
                nc.vector.tensor_scalar(b7[:], s1[:], 7.0, None, op0=ALU.mult)
                mx2 = cpool.tile([PCHUNK, 1], F32, tag="mx2")
                nc.vector.tensor_tensor(mx2[:], z1[:], b7[:], op=ALU.add)
                d2 = cpool.tile([PCHUNK, 1], F32, tag="d2")
                nc.vector.tensor_tensor(d2[:], mx2[:], mn2[:], op=ALU.subtract)
                s2 = cpool.tile([PCHUNK, 1], F32, tag="s2")
                nc.vector.tensor_scalar(s2[:], d2[:], 1e-6, C15,
                                        op0=ALU.max, op1=ALU.mult)

                # out = q*s2 + mn2 (int8 codes read back as fp32)
                OA = opool.tile([PCHUNK, TCOLS], F32, tag="o")
                OB = opool.tile([PCHUNK, TCOLS], F32, tag="o")
                nc.scalar.activation(OA[:], QA[:], ACTF.Identity,
                                     bias=mn2[:, 0:1], scale=s2[:, 0:1])
                nc.scalar.dma_start(out=dst[l0:l0 + PCHUNK, 0:TCOLS], in_=OA[:])
                nc.scalar.activation(OB[:], QB[:], ACTF.Identity,
                                     bias=mn2[:, 0:1], scale=s2[:, 0:1])
                nc.scalar.dma_start(out=dst[l0:l0 + PCHUNK, TCOLS:FD], in_=OB[:])

    nc.compile()
    return nc


def _get_nc(lc=LC):
    if lc not in _BUILD_CACHE:
        _BUILD_CACHE[lc] = _build(lc)
    return _BUILD_CACHE[lc]


def make_in_maps(k_cache_f, v_cache_f):
    """l-major reshard: [B,H,L,D] -> per-core [LC, B*H*D] row blocks."""
    kt = np.ascontiguousarray(np.moveaxis(np.asarray(k_cache_f, np.float32), 2, 0)
                              ).reshape(L, FD)
    vt = np.ascontiguousarray(np.moveaxis(np.asarray(v_cache_f, np.float32), 2, 0)
                              ).reshape(L, FD)
    return [{"k": kt[c * LC:(c + 1) * LC], "v": vt[c * LC:(c + 1) * LC]}
            for c in range(N_CORES)]


def _host_fix_rows(out, cache_idx, val, input_pos):
    """Exact (fp32, reference-op-order) outputs for the scattered rows."""
    f32 = np.float32
    val = np.asarray(val, dtype=np.float32)
    pos = [int(p) for p in np.asarray(input_pos)]
    # last write wins for duplicate positions
    posmap = {}
    for i, p in enumerate(pos):
        posmap[p] = i
    for p, i in posmap.items():
        row = val[:, :, i, :]                       # [B,H,D]
        mn = row.min()
        mx = row.max()
        s2 = f32(max(mx - mn, f32(1e-6)) / f32(15))
        z2 = f32(mn + f32(s2 * f32(8)))
        t = ((row - mn) / s2).astype(np.float32)
        q = np.clip(np.round(t), 0, 15).astype(np.float32)
        out[cache_idx, :, :, p, :] = ((q - f32(8)) * s2).astype(np.float32) + z2


def kernel(k_cache_f, v_cache_f, k_val, v_val, input_pos):
    nc = _get_nc()
    in_maps = make_in_maps(k_cache_f, v_cache_f)
    res = run_bass_kernel_spmd(nc, in_maps, list(range(N_CORES)))
    out = np.empty((2, B, H, L, D), dtype=np.float32)
    for c in range(N_CORES):
        sl = slice(c * LC, (c + 1) * LC)
        out[0, :, :, sl, :] = res.results[c]["ok"].reshape(
            LC, B, H, D).transpose(1, 2, 0, 3)
        out[1, :, :, sl, :] = res.results[c]["ov"].reshape(
            LC, B, H, D).transpose(1, 2, 0, 3)
    _host_fix_rows(out, 0, k_val, input_pos)
    _host_fix_rows(out, 1, v_val, input_pos)
    return out
